# revision 51
# baseline (speedup 1.0000x reference)
"""Trainium2 Bass kernel for nn_CPAMDec_Mix (dual cross-attention decoder block).

Math per batch sample b (C=512, C4=128, K=64, N=W*H=4096):
    pv1 = wv @ y1^T + bv          [C, K]
    pv2 = wv @ y2^T + bv          [C, K]
    q^T = wq @ x2 + bq            [C4, N]
    kk  = y2 @ wk^T + bk          [K, C4]
    energy = q @ kk^T             [N, K]
    att = softmax(|energy|, -1)   [N, K]
    out1 = scale  * pv1 @ att^T + x1
    out2 = scale1 * pv2 @ att^T + x2

Sharding: pure data parallel — sample b on core b (B == n_cores == 8).

Memory-bound problem: all large tensors are staged in DRAM as bf16
(host downcasts inputs / upcasts outputs), ~17 MB HBM traffic per core
(~49 us floor at 358 GB/s). scale/scale1 are folded into the pv
projections so the epilogue is psum + residual.

Schedule: attention quarters and output half-units are interleaved
(q0, q1, cols-0:2048 units, q2, q3, cols-2048:4096 units) so the PE
stream is continuous — the Tensor engine only reaches its full 2.4 GHz
p-state after ~3 us of uninterrupted work, and idle gaps drop it to
1.2 GHz.

The PSUM->SBUF drain of the outputs (4.2M elems) is split across the
two PSUM-capable engines: out1 drains as DVE tensor_tensor adds (+x1);
out2's residual is accumulated in PSUM by the PE via an identity matmul
(I @ x2), so its drain is a plain ACT copy.

DMA: host-packed "SBUF image" layouts (>=2 KB contiguous runs, ~128
descriptors per transfer; descriptor gen is ~5 ns/descriptor on the
issuing engine). Loads split across the sync and scalar HWDGE rings;
all stores issue from the otherwise-idle sync engine.
"""

import numpy as np
import ml_dtypes

import concourse.bass as bass
import concourse.mybir as mybir
import concourse.tile as tile
from concourse import bacc
from concourse.bass_utils import run_bass_kernel_spmd
from concourse.masks import make_identity

F32 = mybir.dt.float32
BF16 = mybir.dt.bfloat16
U32 = mybir.dt.uint32
AX = mybir.AxisListType
OP = mybir.AluOpType
AF = mybir.ActivationFunctionType

B, C, W, H, K = 8, 512, 64, 64, 64
C4 = C // 4          # 128
N = W * H            # 4096
NQ = 1024            # quarter width
CC = C // 128        # 4 chunks of 128 over the channel dim

# packA columns: wq chunks | wk chunks | y2T chunks
WQ0, WK0, Y20 = 0, 512, 1024
WA = 1280
# packB columns: wv chunks | y1T chunks | bv (row 0)
WV0, Y10, BV0 = 0, 2048, 2304
WB = 2816

_CACHE = {}

NPBF16 = ml_dtypes.bfloat16


def _build_nc():
    nc = bacc.Bacc("TRN2", target_bir_lowering=False, debug=False)

    # x2q is quarter-major packed: row q*128+p, col cc*1024+nq maps to
    # x2[cc*128+p, q*1024+nq]. x1/outs are natural [C, N].
    x2q_d = nc.dram_tensor("x2q", [C, N], BF16, kind="ExternalInput")
    x1_d = nc.dram_tensor("x1", [C, N], BF16, kind="ExternalInput")
    packA_d = nc.dram_tensor("packA", [128, WA], BF16, kind="ExternalInput")
    packB_d = nc.dram_tensor("packB", [128, WB], BF16, kind="ExternalInput")
    # per-partition vectors: [bq | bk | scale | scale1]
    vecs_d = nc.dram_tensor("vecs", [C4, 4], F32, kind="ExternalInput")
    out1_d = nc.dram_tensor("out1", [C, N], BF16, kind="ExternalOutput")
    out2_d = nc.dram_tensor("out2", [C, N], BF16, kind="ExternalOutput")

    with tile.TileContext(nc) as tc:
        with (
            tc.tile_pool(name="const", bufs=1) as const,
            tc.tile_pool(name="qpool", bufs=3) as qpool,
            tc.tile_pool(name="spool", bufs=3) as spool,
            tc.tile_pool(name="opool", bufs=10) as opool,
            tc.tile_pool(name="psq", bufs=1, space="PSUM") as psq,
            tc.tile_pool(name="pse", bufs=2, space="PSUM") as pse,
            tc.tile_pool(name="pstp", bufs=1, space="PSUM") as pstp,
            tc.tile_pool(name="pso", bufs=4, space="PSUM") as pso,
        ):
            # ---- loads: vecs first (ACT queue head needs it), then the
            # big tensors split across the two HWDGE rings.
            vecs_sb = const.tile([C4, 4], F32)
            nc.sync.dma_start(out=vecs_sb[:], in_=vecs_d[:])
            packA_sb = const.tile([128, WA], BF16)
            nc.sync.dma_start(out=packA_sb[:], in_=packA_d[:])
            packB_sb = const.tile([128, WB], BF16)
            nc.scalar.dma_start(out=packB_sb[:], in_=packB_d[:])

            # x1 is needed by the first output half (~15 us); x2 q2/q3
            # only by the second attention pair (~40 us). Interleave so
            # x1 never queues behind the late x2 quarters.
            x2_sb = [None] * 4
            x1_sb = [None] * 4

            def load_x2(q, eng):
                t = const.tile([128, CC * NQ], BF16, tag=f"x2_{q}", name="x2t")
                eng.dma_start(out=t[:], in_=x2q_d[q * 128 : (q + 1) * 128, :])
                x2_sb[q] = t

            def load_x1(cc, eng):
                t = const.tile([128, N], BF16, tag=f"x1_{cc}", name="x1t")
                eng.dma_start(out=t[:], in_=x1_d[cc * 128 : (cc + 1) * 128, :])
                x1_sb[cc] = t

            load_x2(0, nc.sync)
            load_x2(1, nc.scalar)
            load_x1(0, nc.sync)
            load_x1(2, nc.scalar)
            load_x2(2, nc.sync)
            load_x1(3, nc.scalar)
            load_x1(1, nc.sync)
            load_x2(3, nc.scalar)

            bq_sb = vecs_sb[:, 0:1]
            bk_sb = vecs_sb[:, 1:2]
            sc_sb = (vecs_sb[0:K, 2:3], vecs_sb[0:K, 3:4])
            bv_sb = packB_sb[0:1, BV0 : BV0 + 512]

            ident = const.tile([128, 128], BF16)
            make_identity(nc, ident[:])
            ones_sb = const.tile([1, K], BF16)
            nc.gpsimd.memset(ones_sb[:], 1.0)

            # ---- HAM warm-up ----
            # The PE clock is gated to 1.2 GHz until the activity monitor
            # sees ~3.4 us of sustained matmul work; bursts shorter than
            # that never release the gate. Stream ~4 us of dummy matmuls
            # (uninitialized SBUF garbage, result never read) while the
            # input DMAs are still in flight, so all real matmuls run at
            # the full 2.4 GHz.
            # ap=128 pulses: Tile deps are program-order counter thresholds,
            # so every dummy cycle sits inside the first softmax op's wait —
            # keep the activity events but minimize their PE time.
            warm_in = const.tile([128, 512], BF16)
            nc.vector.memset(warm_in[:], 1.0)
            pwarm = pso.tile([128, 512], F32, tag="pso", name="pwarm")
            for _ in range(10):
                nc.tensor.matmul(
                    pwarm[:, 0:128],
                    lhsT=warm_in[:, 0:128],
                    rhs=warm_in[:, 0:128],
                    start=True,
                    stop=True,
                )

            # ---- kk^T [C4, K] (needed by every energy matmul) ----
            pkk = pse.tile([C4, K], F32, tag="pse")
            for cc in range(CC):
                nc.tensor.matmul(
                    pkk[:],
                    lhsT=packA_sb[:, WK0 + cc * C4 : WK0 + (cc + 1) * C4],
                    rhs=packA_sb[:, Y20 + cc * K : Y20 + (cc + 1) * K],
                    start=(cc == 0),
                    stop=(cc == CC - 1),
                )
            kkT_sb = const.tile([C4, K], BF16)
            nc.scalar.activation(kkT_sb[:], pkk[:], AF.Identity, bias=bk_sb)

            # ---- pvs_j^T [K, C] = scale_j * (y_j^T.T @ wvT + ones^T bv) ----
            pv_sb = []
            for j, y0 in enumerate((Y10, Y20)):
                ysrc = packB_sb if j == 0 else packA_sb
                ppv = psq.tile([K, C], F32, tag="psq")
                for cc in range(CC):
                    nc.tensor.matmul(
                        ppv[:],
                        lhsT=ysrc[:, y0 + cc * K : y0 + (cc + 1) * K],
                        rhs=packB_sb[:, WV0 + cc * C : WV0 + (cc + 1) * C],
                        start=(cc == 0),
                        stop=False,
                    )
                nc.tensor.matmul(
                    ppv[:], lhsT=ones_sb[:], rhs=bv_sb, start=False, stop=True
                )
                pv = const.tile([K, C], BF16, tag=f"pv_{j}")
                nc.scalar.activation(pv[:], ppv[:], AF.Identity, scale=sc_sb[j])
                pv_sb.append(pv)

            aT_sb = [None] * 4

            def attention_quarter(q):
                """softmax(|q@kk^T|) for cols q*NQ.. -> aT_sb[q] [K, NQ]."""
                pst = pstp.tile([K, NQ], BF16, tag="pst")
                for h in range(2):
                    o = h * 512
                    psum_q = psq.tile([C4, 512], F32, tag="psq")
                    for cc in range(CC):
                        nc.tensor.matmul(
                            psum_q[:],
                            lhsT=packA_sb[:, WQ0 + cc * C4 : WQ0 + (cc + 1) * C4],
                            rhs=x2_sb[q][:, cc * NQ + o : cc * NQ + o + 512],
                            start=(cc == 0),
                            stop=(cc == CC - 1),
                        )
                    qT = qpool.tile([C4, 512], BF16, tag="qT")
                    nc.scalar.activation(qT[:], psum_q[:], AF.Identity, bias=bq_sb)

                    pe = pse.tile([128, 4 * K], F32, tag="pse")
                    for s in range(4):
                        nc.tensor.matmul(
                            pe[:, s * K : (s + 1) * K],
                            lhsT=qT[:, s * 128 : (s + 1) * 128],
                            rhs=kkT_sb[:],
                            start=True,
                            stop=True,
                        )
                    # softmax(|e|) along k, no max-subtraction (|e| <~ 20)
                    eexp = spool.tile([128, 4 * K], F32, tag="eexp")
                    nc.vector.tensor_scalar(
                        eexp[:].bitcast(U32),
                        pe[:].bitcast(U32),
                        0x7FFFFFFF,
                        None,
                        op0=OP.bitwise_and,
                    )
                    nc.scalar.activation(eexp[:], eexp[:], AF.Exp)
                    rsum = spool.tile([128, 4], F32, tag="rsum")
                    nc.vector.tensor_reduce(
                        rsum[:],
                        eexp[:].rearrange("p (g d) -> p g d", g=4),
                        axis=AX.X,
                        op=OP.add,
                    )
                    rrec = spool.tile([128, 4], F32, tag="rrec")
                    nc.vector.reciprocal(rrec[:], rsum[:])
                    att = spool.tile([128, 4 * K], BF16, tag="att")
                    nc.gpsimd.tensor_tensor(
                        att[:].rearrange("p (g d) -> p g d", g=4),
                        eexp[:].rearrange("p (g d) -> p g d", g=4),
                        rrec[:].to_broadcast((128, 4, K)),
                        op=OP.mult,
                    )
                    for s in range(4):
                        nc.tensor.transpose(
                            pst[:, o + s * 128 : o + (s + 1) * 128],
                            att[:, s * K : (s + 1) * K],
                            ident[:],
                        )
                aT = const.tile([K, NQ], BF16, tag=f"aT_{q}")
                nc.vector.tensor_copy(aT[:], pst[:])
                aT_sb[q] = aT

            def output_half(half):
                """units (cc, j) for cols half*2048 .. half*2048+2048.
                Each [128, NQ] slab stores out as soon as its two PSUM
                pieces are drained — no whole-unit aggregation, so the
                store stream never waits on a 4-drain convoy."""
                for cc in range(CC):
                    for j in range(2):
                        for qq in range(2):
                            q = half * 2 + qq
                            o_t = opool.tile([128, NQ], BF16, tag="o")
                            for h in range(2):
                                po = pso.tile([128, 512], F32, tag="pso")
                                nc.tensor.matmul(
                                    po[:],
                                    lhsT=pv_sb[j][:, cc * 128 : (cc + 1) * 128],
                                    rhs=aT_sb[q][:, h * 512 : (h + 1) * 512],
                                    start=True,
                                    stop=(j == 0),
                                )
                                if j == 1:
                                    nc.tensor.matmul(
                                        po[:],
                                        lhsT=ident[:],
                                        rhs=x2_sb[q][
                                            :,
                                            cc * NQ + h * 512 : cc * NQ + h * 512 + 512,
                                        ],
                                        start=False,
                                        stop=True,
                                    )
                                osl = o_t[:, h * 512 : h * 512 + 512]
                                if j == 0:
                                    nc.vector.tensor_tensor(
                                        osl,
                                        po[:],
                                        x1_sb[cc][
                                            :,
                                            q * NQ + h * 512 : q * NQ + h * 512 + 512,
                                        ],
                                        op=OP.add,
                                    )
                                else:
                                    nc.scalar.copy(osl, po[:])
                            out_d = out1_d if j == 0 else out2_d
                            # second half: the scalar ring is idle (loads
                            # done), so give it the out2 stores to double
                            # the store-completion pipeline in the tail.
                            eng = nc.scalar if (half == 1 and j == 1) else nc.sync
                            eng.dma_start(
                                out=out_d[
                                    cc * 128 : (cc + 1) * 128,
                                    q * NQ : (q + 1) * NQ,
                                ],
                                in_=o_t[:],
                            )

            attention_quarter(0)
            attention_quarter(1)
            output_half(0)
            attention_quarter(2)
            attention_quarter(3)
            output_half(1)
    nc.compile()
    return nc


def _get_nc():
    if "nc" not in _CACHE:
        _CACHE["nc"] = _build_nc()
    return _CACHE["nc"]


def _chunk(a):
    """[C, D] -> [128, CC*D] SBUF image (row chunk cc at cols cc*D..)."""
    d = a.shape[1]
    return np.ascontiguousarray(
        a.reshape(CC, 128, d).transpose(1, 0, 2).reshape(128, CC * d)
    )


def kernel(x1, y1, x2, y2, wq, bq, wk, bk, wv, bv, scale, scale1, **run_kwargs):
    x1 = np.asarray(x1, np.float32).reshape(B, C, N)
    x2 = np.asarray(x2, np.float32).reshape(B, C, N)
    y1 = np.asarray(y1, np.float32)
    y2 = np.asarray(y2, np.float32)
    wq = np.asarray(wq, np.float32)
    wk = np.asarray(wk, np.float32)
    wv = np.asarray(wv, np.float32)

    vecs = np.stack(
        [
            np.asarray(bq, np.float32).reshape(C4),
            np.asarray(bk, np.float32).reshape(C4),
            np.full(C4, np.asarray(scale).reshape(-1)[0], np.float32),
            np.full(C4, np.asarray(scale1).reshape(-1)[0], np.float32),
        ],
        axis=1,
    )
    vecs = np.ascontiguousarray(vecs)

    packA_shared = np.concatenate(
        [_chunk(wq.T.astype(NPBF16)), _chunk(wk.T.astype(NPBF16))], axis=1
    )
    bvblk = np.zeros((128, 512), NPBF16)
    bvblk[0, :] = np.asarray(bv, np.float32).reshape(C).astype(NPBF16)
    packB_shared = _chunk(wv.T.astype(NPBF16))

    in_maps = []
    for b in range(B):
        packA = np.concatenate(
            [packA_shared, _chunk(y2[b].T.astype(NPBF16))], axis=1
        )
        packB = np.concatenate(
            [packB_shared, _chunk(y1[b].T.astype(NPBF16)), bvblk], axis=1
        )
        x2qb = np.ascontiguousarray(
            x2[b]
            .astype(NPBF16)
            .reshape(CC, 128, 4, NQ)
            .transpose(2, 1, 0, 3)
            .reshape(C, N)
        )
        in_maps.append(
            {
                "x2q": x2qb,
                "x1": np.ascontiguousarray(x1[b].astype(NPBF16)),
                "packA": np.ascontiguousarray(packA),
                "packB": np.ascontiguousarray(packB),
                "vecs": vecs,
            }
        )
    nc = _get_nc()
    res = run_bass_kernel_spmd(nc, in_maps, list(range(B)), **run_kwargs)
    _CACHE["last_results"] = res
    out1 = np.stack(
        [
            np.asarray(res.results[b]["out1"]).astype(np.float32).reshape(C, W, H)
            for b in range(B)
        ]
    )
    out2 = np.stack(
        [
            np.asarray(res.results[b]["out2"]).astype(np.float32).reshape(C, W, H)
            for b in range(B)
        ]
    )
    return (out1, out2)


# revision 52
# speedup vs baseline: 1.0085x; 1.0085x over previous
"""Trainium2 Bass kernel for nn_CPAMDec_Mix (dual cross-attention decoder block).

Math per batch sample b (C=512, C4=128, K=64, N=W*H=4096):
    pv1 = wv @ y1^T + bv          [C, K]
    pv2 = wv @ y2^T + bv          [C, K]
    q^T = wq @ x2 + bq            [C4, N]
    kk  = y2 @ wk^T + bk          [K, C4]
    energy = q @ kk^T             [N, K]
    att = softmax(|energy|, -1)   [N, K]
    out1 = scale  * pv1 @ att^T + x1
    out2 = scale1 * pv2 @ att^T + x2

Sharding: pure data parallel — sample b on core b (B == n_cores == 8).

Memory-bound problem: all large tensors are staged in DRAM as bf16
(host downcasts inputs / upcasts outputs), ~17 MB HBM traffic per core
(~49 us floor at 358 GB/s). scale/scale1 are folded into the pv
projections so the epilogue is psum + residual.

Schedule: attention quarters and output half-units are interleaved
(q0, q1, cols-0:2048 units, q2, q3, cols-2048:4096 units) so the PE
stream is continuous — the Tensor engine only reaches its full 2.4 GHz
p-state after ~3 us of uninterrupted work, and idle gaps drop it to
1.2 GHz.

The PSUM->SBUF drain of the outputs (4.2M elems) is split across the
two PSUM-capable engines: out1 drains as DVE tensor_tensor adds (+x1);
out2's residual is accumulated in PSUM by the PE via an identity matmul
(I @ x2), so its drain is a plain ACT copy.

DMA: host-packed "SBUF image" layouts (>=2 KB contiguous runs, ~128
descriptors per transfer; descriptor gen is ~5 ns/descriptor on the
issuing engine). Loads split across the sync and scalar HWDGE rings;
all stores issue from the otherwise-idle sync engine.
"""

import numpy as np
import ml_dtypes

import concourse.bass as bass
import concourse.mybir as mybir
import concourse.tile as tile
from concourse import bacc
from concourse.bass_utils import run_bass_kernel_spmd
from concourse.masks import make_identity

F32 = mybir.dt.float32
BF16 = mybir.dt.bfloat16
U32 = mybir.dt.uint32
AX = mybir.AxisListType
OP = mybir.AluOpType
AF = mybir.ActivationFunctionType

B, C, W, H, K = 8, 512, 64, 64, 64
C4 = C // 4          # 128
N = W * H            # 4096
NQ = 1024            # quarter width
CC = C // 128        # 4 chunks of 128 over the channel dim

# packA columns: wq chunks | wk chunks | y2T chunks
WQ0, WK0, Y20 = 0, 512, 1024
WA = 1280
# packB columns: wv chunks | y1T chunks | bv (row 0)
WV0, Y10, BV0 = 0, 2048, 2304
WB = 2816

_CACHE = {}

NPBF16 = ml_dtypes.bfloat16


def _build_nc():
    nc = bacc.Bacc("TRN2", target_bir_lowering=False, debug=False)

    # x2q is quarter-major packed: row q*128+p, col cc*1024+nq maps to
    # x2[cc*128+p, q*1024+nq]. x1/outs are natural [C, N].
    x2q_d = nc.dram_tensor("x2q", [C, N], BF16, kind="ExternalInput")
    x1_d = nc.dram_tensor("x1", [C, N], BF16, kind="ExternalInput")
    packA_d = nc.dram_tensor("packA", [128, WA], BF16, kind="ExternalInput")
    packB_d = nc.dram_tensor("packB", [128, WB], BF16, kind="ExternalInput")
    # per-partition vectors: [bq | bk | scale | scale1]
    vecs_d = nc.dram_tensor("vecs", [C4, 4], F32, kind="ExternalInput")
    out1_d = nc.dram_tensor("out1", [C, N], BF16, kind="ExternalOutput")
    out2_d = nc.dram_tensor("out2", [C, N], BF16, kind="ExternalOutput")

    with tile.TileContext(nc) as tc:
        with (
            tc.tile_pool(name="const", bufs=1) as const,
            tc.tile_pool(name="qpool", bufs=3) as qpool,
            tc.tile_pool(name="spool", bufs=3) as spool,
            tc.tile_pool(name="opool", bufs=10) as opool,
            tc.tile_pool(name="psq", bufs=1, space="PSUM") as psq,
            tc.tile_pool(name="pse", bufs=2, space="PSUM") as pse,
            tc.tile_pool(name="pstp", bufs=1, space="PSUM") as pstp,
            tc.tile_pool(name="pso", bufs=4, space="PSUM") as pso,
        ):
            # ---- loads: vecs first (ACT queue head needs it), then the
            # big tensors split across the two HWDGE rings.
            vecs_sb = const.tile([C4, 4], F32)
            nc.sync.dma_start(out=vecs_sb[:], in_=vecs_d[:])
            packA_sb = const.tile([128, WA], BF16)
            nc.sync.dma_start(out=packA_sb[:], in_=packA_d[:])
            packB_sb = const.tile([128, WB], BF16)
            nc.scalar.dma_start(out=packB_sb[:], in_=packB_d[:])

            # x1 is needed by the first output half (~15 us); x2 q2/q3
            # only by the second attention pair (~40 us). Interleave so
            # x1 never queues behind the late x2 quarters.
            x2_sb = [None] * 4
            x1_sb = [None] * 4

            def load_x2(q, eng):
                t = const.tile([128, CC * NQ], BF16, tag=f"x2_{q}", name="x2t")
                eng.dma_start(out=t[:], in_=x2q_d[q * 128 : (q + 1) * 128, :])
                x2_sb[q] = t

            def load_x1(cc, eng):
                t = const.tile([128, N], BF16, tag=f"x1_{cc}", name="x1t")
                eng.dma_start(out=t[:], in_=x1_d[cc * 128 : (cc + 1) * 128, :])
                x1_sb[cc] = t

            load_x2(0, nc.sync)
            load_x2(1, nc.scalar)
            load_x1(0, nc.sync)
            load_x1(2, nc.scalar)
            load_x2(2, nc.sync)
            load_x1(3, nc.scalar)
            load_x1(1, nc.sync)
            load_x2(3, nc.scalar)

            bq_sb = vecs_sb[:, 0:1]
            bk_sb = vecs_sb[:, 1:2]
            sc_sb = (vecs_sb[0:K, 2:3], vecs_sb[0:K, 3:4])
            bv_sb = packB_sb[0:1, BV0 : BV0 + 512]

            ident = const.tile([128, 128], BF16)
            make_identity(nc, ident[:])
            ones_sb = const.tile([1, K], BF16)
            nc.gpsimd.memset(ones_sb[:], 1.0)

            # ---- HAM warm-up ----
            # The PE clock is gated to 1.2 GHz until the activity monitor
            # sees ~3.4 us of sustained matmul work; bursts shorter than
            # that never release the gate. Stream ~4 us of dummy matmuls
            # (uninitialized SBUF garbage, result never read) while the
            # input DMAs are still in flight, so all real matmuls run at
            # the full 2.4 GHz.
            warm_in = const.tile([128, 512], BF16)
            nc.vector.memset(warm_in[:], 1.0)
            pwarm = pso.tile([128, 512], F32, tag="pso", name="pwarm")
            for _ in range(10):
                nc.tensor.matmul(
                    pwarm[:],
                    lhsT=warm_in[:, 0:128],
                    rhs=warm_in[:],
                    start=True,
                    stop=True,
                )

            # ---- kk^T [C4, K] (needed by every energy matmul) ----
            pkk = pse.tile([C4, K], F32, tag="pse")
            for cc in range(CC):
                nc.tensor.matmul(
                    pkk[:],
                    lhsT=packA_sb[:, WK0 + cc * C4 : WK0 + (cc + 1) * C4],
                    rhs=packA_sb[:, Y20 + cc * K : Y20 + (cc + 1) * K],
                    start=(cc == 0),
                    stop=(cc == CC - 1),
                )
            kkT_sb = const.tile([C4, K], BF16)
            nc.scalar.activation(kkT_sb[:], pkk[:], AF.Identity, bias=bk_sb)

            # ---- pvs_j^T [K, C] = scale_j * (y_j^T.T @ wvT + ones^T bv) ----
            pv_sb = []
            for j, y0 in enumerate((Y10, Y20)):
                ysrc = packB_sb if j == 0 else packA_sb
                ppv = psq.tile([K, C], F32, tag="psq")
                for cc in range(CC):
                    nc.tensor.matmul(
                        ppv[:],
                        lhsT=ysrc[:, y0 + cc * K : y0 + (cc + 1) * K],
                        rhs=packB_sb[:, WV0 + cc * C : WV0 + (cc + 1) * C],
                        start=(cc == 0),
                        stop=False,
                    )
                nc.tensor.matmul(
                    ppv[:], lhsT=ones_sb[:], rhs=bv_sb, start=False, stop=True
                )
                pv = const.tile([K, C], BF16, tag=f"pv_{j}")
                nc.scalar.activation(pv[:], ppv[:], AF.Identity, scale=sc_sb[j])
                pv_sb.append(pv)

            aT_sb = [None] * 4

            def attention_quarter(q):
                """softmax(|q@kk^T|) for cols q*NQ.. -> aT_sb[q] [K, NQ]."""
                pst = pstp.tile([K, NQ], BF16, tag="pst")
                for h in range(2):
                    o = h * 512
                    psum_q = psq.tile([C4, 512], F32, tag="psq")
                    for cc in range(CC):
                        nc.tensor.matmul(
                            psum_q[:],
                            lhsT=packA_sb[:, WQ0 + cc * C4 : WQ0 + (cc + 1) * C4],
                            rhs=x2_sb[q][:, cc * NQ + o : cc * NQ + o + 512],
                            start=(cc == 0),
                            stop=(cc == CC - 1),
                        )
                    qT = qpool.tile([C4, 512], BF16, tag="qT")
                    nc.scalar.activation(qT[:], psum_q[:], AF.Identity, bias=bq_sb)

                    pe = pse.tile([128, 4 * K], F32, tag="pse")
                    for s in range(4):
                        nc.tensor.matmul(
                            pe[:, s * K : (s + 1) * K],
                            lhsT=qT[:, s * 128 : (s + 1) * 128],
                            rhs=kkT_sb[:],
                            start=True,
                            stop=True,
                        )
                    # softmax(|e|) along k, no max-subtraction (|e| <~ 20)
                    eexp = spool.tile([128, 4 * K], F32, tag="eexp")
                    nc.vector.tensor_scalar(
                        eexp[:].bitcast(U32),
                        pe[:].bitcast(U32),
                        0x7FFFFFFF,
                        None,
                        op0=OP.bitwise_and,
                    )
                    nc.scalar.activation(eexp[:], eexp[:], AF.Exp)
                    rsum = spool.tile([128, 4], F32, tag="rsum")
                    nc.vector.tensor_reduce(
                        rsum[:],
                        eexp[:].rearrange("p (g d) -> p g d", g=4),
                        axis=AX.X,
                        op=OP.add,
                    )
                    rrec = spool.tile([128, 4], F32, tag="rrec")
                    nc.vector.reciprocal(rrec[:], rsum[:])
                    att = spool.tile([128, 4 * K], BF16, tag="att")
                    nc.gpsimd.tensor_tensor(
                        att[:].rearrange("p (g d) -> p g d", g=4),
                        eexp[:].rearrange("p (g d) -> p g d", g=4),
                        rrec[:].to_broadcast((128, 4, K)),
                        op=OP.mult,
                    )
                    for s in range(4):
                        nc.tensor.transpose(
                            pst[:, o + s * 128 : o + (s + 1) * 128],
                            att[:, s * K : (s + 1) * K],
                            ident[:],
                        )
                aT = const.tile([K, NQ], BF16, tag=f"aT_{q}")
                nc.vector.tensor_copy(aT[:], pst[:])
                aT_sb[q] = aT

            def output_half(half):
                """units (cc, j) for cols half*2048 .. half*2048+2048.
                Each [128, NQ] slab stores out as soon as its two PSUM
                pieces are drained — no whole-unit aggregation, so the
                store stream never waits on a 4-drain convoy."""
                for cc in range(CC):
                    for j in range(2):
                        for qq in range(2):
                            q = half * 2 + qq
                            o_t = opool.tile([128, NQ], BF16, tag="o")
                            for h in range(2):
                                po = pso.tile([128, 512], F32, tag="pso")
                                nc.tensor.matmul(
                                    po[:],
                                    lhsT=pv_sb[j][:, cc * 128 : (cc + 1) * 128],
                                    rhs=aT_sb[q][:, h * 512 : (h + 1) * 512],
                                    start=True,
                                    stop=(j == 0),
                                )
                                if j == 1:
                                    nc.tensor.matmul(
                                        po[:],
                                        lhsT=ident[:],
                                        rhs=x2_sb[q][
                                            :,
                                            cc * NQ + h * 512 : cc * NQ + h * 512 + 512,
                                        ],
                                        start=False,
                                        stop=True,
                                    )
                                osl = o_t[:, h * 512 : h * 512 + 512]
                                if j == 0:
                                    nc.vector.tensor_tensor(
                                        osl,
                                        po[:],
                                        x1_sb[cc][
                                            :,
                                            q * NQ + h * 512 : q * NQ + h * 512 + 512,
                                        ],
                                        op=OP.add,
                                    )
                                else:
                                    nc.scalar.copy(osl, po[:])
                            out_d = out1_d if j == 0 else out2_d
                            # second half: the scalar ring is idle (loads
                            # done), so give it the out2 stores to double
                            # the store-completion pipeline in the tail.
                            eng = nc.scalar if (half == 1 and j == 1) else nc.sync
                            eng.dma_start(
                                out=out_d[
                                    cc * 128 : (cc + 1) * 128,
                                    q * NQ : (q + 1) * NQ,
                                ],
                                in_=o_t[:],
                            )

            attention_quarter(0)
            attention_quarter(1)
            output_half(0)
            attention_quarter(2)
            attention_quarter(3)
            output_half(1)
    nc.compile()
    return nc


def _get_nc():
    if "nc" not in _CACHE:
        _CACHE["nc"] = _build_nc()
    return _CACHE["nc"]


def _chunk(a):
    """[C, D] -> [128, CC*D] SBUF image (row chunk cc at cols cc*D..)."""
    d = a.shape[1]
    return np.ascontiguousarray(
        a.reshape(CC, 128, d).transpose(1, 0, 2).reshape(128, CC * d)
    )


def kernel(x1, y1, x2, y2, wq, bq, wk, bk, wv, bv, scale, scale1, **run_kwargs):
    x1 = np.asarray(x1, np.float32).reshape(B, C, N)
    x2 = np.asarray(x2, np.float32).reshape(B, C, N)
    y1 = np.asarray(y1, np.float32)
    y2 = np.asarray(y2, np.float32)
    wq = np.asarray(wq, np.float32)
    wk = np.asarray(wk, np.float32)
    wv = np.asarray(wv, np.float32)

    vecs = np.stack(
        [
            np.asarray(bq, np.float32).reshape(C4),
            np.asarray(bk, np.float32).reshape(C4),
            np.full(C4, np.asarray(scale).reshape(-1)[0], np.float32),
            np.full(C4, np.asarray(scale1).reshape(-1)[0], np.float32),
        ],
        axis=1,
    )
    vecs = np.ascontiguousarray(vecs)

    packA_shared = np.concatenate(
        [_chunk(wq.T.astype(NPBF16)), _chunk(wk.T.astype(NPBF16))], axis=1
    )
    bvblk = np.zeros((128, 512), NPBF16)
    bvblk[0, :] = np.asarray(bv, np.float32).reshape(C).astype(NPBF16)
    packB_shared = _chunk(wv.T.astype(NPBF16))

    in_maps = []
    for b in range(B):
        packA = np.concatenate(
            [packA_shared, _chunk(y2[b].T.astype(NPBF16))], axis=1
        )
        packB = np.concatenate(
            [packB_shared, _chunk(y1[b].T.astype(NPBF16)), bvblk], axis=1
        )
        x2qb = np.ascontiguousarray(
            x2[b]
            .astype(NPBF16)
            .reshape(CC, 128, 4, NQ)
            .transpose(2, 1, 0, 3)
            .reshape(C, N)
        )
        in_maps.append(
            {
                "x2q": x2qb,
                "x1": np.ascontiguousarray(x1[b].astype(NPBF16)),
                "packA": np.ascontiguousarray(packA),
                "packB": np.ascontiguousarray(packB),
                "vecs": vecs,
            }
        )
    nc = _get_nc()
    res = run_bass_kernel_spmd(nc, in_maps, list(range(B)), **run_kwargs)
    _CACHE["last_results"] = res
    out1 = np.stack(
        [
            np.asarray(res.results[b]["out1"]).astype(np.float32).reshape(C, W, H)
            for b in range(B)
        ]
    )
    out2 = np.stack(
        [
            np.asarray(res.results[b]["out2"]).astype(np.float32).reshape(C, W, H)
            for b in range(B)
        ]
    )
    return (out1, out2)
